# revision 11
# baseline (speedup 1.0000x reference)
"""Trainium2 Bass kernel for nn_CrossAttention_74818330296332.

Reference computation (per batch b):
  q   = Conv1x1(x, Wq)                          # [C, H, W]
  kv  = Conv3x3_same(condition, Wkv) + bkv      # [2C, H, W]
  k, v = split(kv)                              # each [C, H, W]
  S   = q @ k^T over W (per channel)            # [C, H, H]
  A   = softmax(S * C**-0.5, axis=-1)
  att = A @ v                                   # [C, H, W]
  out = Conv1x1(att, Wproj) + bproj + x

Sharding: data-parallel over batch B=8 across the 8 NeuronCores.

Per-core pipeline (fp32 PSUM accumulate everywhere):
  Host prep: cond/x pre-cast to fp8 (so all device loads are plain HWDGE
    DMAs, no software-DGE casting), weights pre-scaled by 64 into fp8.
  Phase B: 3x3 conv as 9 shifted 1x1 matmuls accumulated in PSUM in
    fp8 DoubleRow (K=256 per matmul).  Spatial chunks of 32 rows with
    1-row halo; q conv (fp8, non-DR) rides along.  Writes q/k/v fp8
    [C, H, W] to DRAM scratch.
  Phase C (attention, per channel-group of 8):  Q^T/K^T loaded via DMA
    transpose of the fp8 scratch viewed as uint16 (so each transposed
    element carries a (w, w+1) fp8 pair == the DoubleRow kt pair);
    K^T repacked on DVE so its kt stride is 16B-aligned for LDWEIGHTS.
    S^T = K Q^T with one DR matmul per 128-g block; exp via ACT into
    fp8; A@v with one DR matmul per 128-h block against V tiles that
    carry a ones-column, so the softmax denominator falls out of the
    same matmul (no FD=1 denominator matmuls); normalization is a
    per-partition scalar multiply split across DVE and GpSimd.
  Phase D: 1x1 proj conv (fp8) + bias + bf16 residual add, streaming.
"""

import os
import sys
import types

import numpy as np
import ml_dtypes

# Make NTFF tracing available if requested (no-op for plain runs).
try:
    import antenv

    if not hasattr(antenv, "axon_hooks"):
        _m = types.ModuleType("antenv.axon_hooks")
        _hook = [None]
        _m.set_axon_ntff_profile_hook = lambda h: _hook.__setitem__(0, h)
        _m.get_axon_ntff_profile_hook = lambda: _hook[0]
        sys.modules["antenv.axon_hooks"] = _m
        antenv.axon_hooks = _m
except Exception:
    pass

import concourse.bass as bass  # noqa: E402
import concourse.tile as tile  # noqa: E402
from concourse import bacc, mybir  # noqa: E402
from concourse.bass_utils import run_bass_kernel_spmd  # noqa: E402

BF16 = mybir.dt.bfloat16
F32 = mybir.dt.float32
FP8 = mybir.dt.float8e4
U16 = mybir.dt.uint16
PM = mybir.MatmulPerfMode
AFT = mybir.ActivationFunctionType

B, C, C_COND, H, W = 8, 128, 256, 256, 256
HW = H * W
SCALE = float(C) ** -0.5
WSCALE = 64.0         # fp8 weight pre-scale (undone in PSUM evacuations)

def _mm_noldw(te, out, rhs, start, stop, perf_mode=None):
    """InstMatmult with no stationary operand: walrus emits no LDWEIGHTS.
    Must directly follow a te.ldweights() of the weights it should use."""
    keep_dims = {0}
    if perf_mode in (mybir.MatmulPerfMode.DoubleRow,
                     mybir.MatmulPerfMode.DoubleRowSwInterleave):
        keep_dims.add(1)
    ifmap_ap = te.lower_ap(rhs.opt(keep_dims), opt=False)
    out_ap = te.lower_ap(out)
    return te.add_instruction(
        mybir.InstMatmult(
            name=te.bass.get_next_instruction_name(),
            replication_resolution=0,
            replication_shift_amnt=0,
            replication_num_rows=0,
            start_tensor_calc=start,
            stop_tensor_calc=stop,
            ins=[ifmap_ap],
            outs=[out_ap],
            perf_mode=perf_mode,
            is_transpose=False,
            ifmap_quant_offset=None,
            weights_quant_offset=None,
            bass_skip_group_check=True,
            tile_position=(0, 0),
            tile_size=(128, 128),
        ))


N_CHUNKS = 8          # phase B spatial chunks
RC = H // N_CHUNKS    # rows per chunk (32)
CW = W + 16           # padded row length (272; %16 for DoubleRow AP strides)
N_GROUPS = 16         # phase C channel groups
GC = C // N_GROUPS    # channels per group (8)
D_CHUNK = 2048        # phase D pixels per chunk


def _emit(tc):
    nc = tc.nc

    x8_d = nc.dram_tensor("x8", [C, HW], FP8, kind="ExternalInput").ap()
    xbf_d = nc.dram_tensor("xbf", [C, HW], BF16, kind="ExternalInput").ap()
    cond_d = nc.dram_tensor("cond8", [C_COND, H, W], FP8,
                            kind="ExternalInput").ap()
    wq_d = nc.dram_tensor("wq8", [C, C], FP8, kind="ExternalInput").ap()
    wkv_d = nc.dram_tensor("wkv", [128, 18, 2, 128], FP8,
                           kind="ExternalInput").ap()
    bkv_d = nc.dram_tensor("bkv", [128, 2], F32, kind="ExternalInput").ap()
    wproj_d = nc.dram_tensor("wproj8", [C, C], FP8, kind="ExternalInput").ap()
    bproj_d = nc.dram_tensor("bproj", [C, 1], F32, kind="ExternalInput").ap()

    q_t = nc.dram_tensor("q_s", [C, H, W], FP8, kind="Internal")
    k_t = nc.dram_tensor("k_s", [C, H, W], FP8, kind="Internal")
    v_t = nc.dram_tensor("v_s", [C, H, W], FP8, kind="Internal")
    att_t = nc.dram_tensor("att_s", [C, HW], FP8, kind="Internal")
    q_d, k_d, v_d, att_d = q_t.ap(), k_t.ap(), v_t.ap(), att_t.ap()
    q16 = q_t.bitcast(U16).ap()     # [C, H, W//2]
    k16 = k_t.bitcast(U16).ap()
    out_d = nc.dram_tensor("out", [C, HW], F32, kind="ExternalOutput").ap()

    q_f = q_d.rearrange("c h w -> c (h w)")
    k_f = k_d.rearrange("c h w -> c (h w)")
    v_f = v_d.rearrange("c h w -> c (h w)")

    # ---------------- globals ----------------
    with tc.tile_pool(name="glob", bufs=1) as glob:
        wproj_sb = glob.tile([128, 128], FP8)
        nc.sync.dma_start(wproj_sb[:], wproj_d[:])
        bproj_sb = glob.tile([128, 1], F32)
        nc.sync.dma_start(bproj_sb[:], bproj_d[:])

        # ---------------- phase B: q conv (fp8) + kv conv (fp8 DoubleRow) ----
        with tc.tile_pool(name="pb_const", bufs=1) as pbc, \
             tc.tile_pool(name="pb_ps", bufs=2, space="PSUM") as cvp, \
             tc.tile_pool(name="pb_stage", bufs=3) as stp, \
             tc.tile_pool(name="pb_x", bufs=2) as xp:
            wq_sb = pbc.tile([128, 128], FP8)
            nc.sync.dma_start(wq_sb[:], wq_d[:])
            # [i, t=(ob,dy,dx), kt=ib, o] fp8, pre-scaled by WSCALE
            wkv_sb = pbc.tile([128, 18, 2, 128], FP8)
            nc.sync.dma_start(wkv_sb[:], wkv_d[:])
            bkv_sb = pbc.tile([128, 2], F32)
            nc.sync.dma_start(bkv_sb[:], bkv_d[:])

            # persistent A/B cond tiles: [128, kt=ib, 34 rows, 272 cols] fp8
            # with zero pad columns 0 and 257.. (w padding of the SAME conv)
            ct = [pbc.tile([128, 2, RC + 2, CW], FP8, name=f"ct{p}")
                  for p in range(2)]
            for p in range(2):
                nc.vector.memset(ct[p][:, :, :, 0:1], 0.0)
                nc.vector.memset(ct[p][:, :, :, W + 1:CW], 0.0)

            for chunk in range(N_CHUNKS):
                r0 = chunk * RC
                par = chunk % 2
                t = ct[par]
                # load cond rows [r0-1, r0+RC+1) with edge clipping
                lo = r0 - 1
                hi = r0 + RC + 1
                tlo = 0
                if lo < 0:
                    nc.vector.memset(t[:, :, 0:1, :], 0.0)
                    lo, tlo = 0, 1
                if hi > H:
                    nc.vector.memset(t[:, :, RC + 1:RC + 2, :], 0.0)
                    hi = H
                for ib in range(2):
                    eng = nc.sync if ib == 0 else nc.scalar
                    eng.dma_start(
                        out=t[:, ib, tlo:tlo + (hi - lo), 1:W + 1],
                        in_=cond_d[ib * 128:(ib + 1) * 128, lo:hi, :])

                # kv conv: 4 quads of 2048 px; per tap one weight feeds 4 MMs
                for quad in range(4):
                    for ob in range(2):
                        ps = cvp.tile([128, 2048], F32, name=f"cv{ob}",
                                      tag="convps")
                        for dy in range(3):
                            for dx in range(3):
                                ti = ob * 9 + dy * 3 + dx
                                for s in range(4):
                                    rr = 8 * quad + 2 * s + dy
                                    nc.tensor.matmul(
                                        ps[:, s * 512:(s + 1) * 512],
                                        lhsT=wkv_sb[:, ti, :, :],
                                        rhs=t[:, :, rr:rr + 2, dx:dx + W],
                                        start=(dy == 0 and dx == 0),
                                        stop=(dy == 2 and dx == 2),
                                        perf_mode=PM.DoubleRow,
                                        skip_group_check=True)
                        kvst = stp.tile([128, 2048], FP8, name="kvst")
                        nc.scalar.activation(kvst[:], ps[:], func=AFT.Identity,
                                             bias=bkv_sb[:, ob:ob + 1],
                                             scale=1.0 / WSCALE)
                        dst = k_f if ob == 0 else v_f
                        off = r0 * W + quad * 2048
                        nc.sync.dma_start(dst[:, off:off + 2048], kvst[:])

                # q conv for the same 32 rows, two halves of 16 rows
                for half in range(2):
                    off = (r0 + 16 * half) * W
                    xt = xp.tile([128, 4096], FP8, name="xt")
                    nc.scalar.dma_start(out=xt[:], in_=x8_d[:, off:off + 4096])
                    qst = stp.tile([128, 4096], FP8, name="qst")
                    for j2 in range(2):
                        qps = cvp.tile([128, 2048], F32, name="qps",
                                       tag="convps")
                        for s in range(4):
                            j = j2 * 4 + s
                            nc.tensor.matmul(
                                qps[:, s * 512:(s + 1) * 512],
                                lhsT=wq_sb[:],
                                rhs=xt[:, j * 512:(j + 1) * 512],
                                start=True, stop=True)
                        nc.vector.tensor_scalar_mul(
                            qst[:, j2 * 2048:(j2 + 1) * 2048], qps[:],
                            1.0 / WSCALE)
                    nc.sync.dma_start(q_f[:, off:off + 4096], qst[:])

        # ---------------- phase C: per-channel attention ----------------
        att_v = att_d.rearrange("c (h w) -> c h w", h=H)
        with tc.tile_pool(name="pc_in", bufs=3) as pci, \
             tc.tile_pool(name="pc_est", bufs=3) as pce, \
             tc.tile_pool(name="pc_r", bufs=3) as pcr, \
             tc.tile_pool(name="pc_ao", bufs=2) as pao, \
             tc.tile_pool(name="pc_stps", bufs=2, space="PSUM") as stps, \
             tc.tile_pool(name="pc_aps", bufs=3, space="PSUM") as aps:
            for g in range(N_GROUPS):
                c0 = g * GC
                # Q^T / K^T via uint16 xbar transpose: each element is an
                # fp8 (w, w+1) pair == the DoubleRow kt pair.
                qt16 = pci.tile([128, GC, H], U16, name="qt16")
                nc.sync.dma_start(
                    out=qt16[:],
                    in_=q16[c0:c0 + GC, :, :].rearrange("c h w -> (c h) w"),
                    transpose=True)
                kt16 = pci.tile([128, GC, H], U16, name="kt16")
                nc.sync.dma_start(
                    out=kt16[:],
                    in_=k16[c0:c0 + GC, :, :].rearrange("c h w -> (c h) w"),
                    transpose=True)
                # V tiles [g0, c, gb, w+ones]: plain permuted loads
                vt = pci.tile([128, GC, 2, W + 1], FP8, name="vt")
                nc.vector.memset(vt[:, :, :, W:W + 1], 1.0)
                for gb in range(2):
                    eng = nc.sync if gb == 0 else nc.scalar
                    eng.dma_start(
                        out=vt[:, :, gb, 0:W],
                        in_=v_d[c0:c0 + GC, gb * 128:(gb + 1) * 128, :]
                        .rearrange("c h w -> h c w"))
                ao = [pao.tile([128, GC, W], FP8, name=f"ao{hb}")
                      for hb in range(2)]

                # fp8 views of the transposed tiles: partition = w-pair,
                # innermost t = w parity (stride 1)
                qt8 = qt16[:].rearrange("p c h -> p (c h)").bitcast(FP8) \
                    .rearrange("p (c h t) -> p c h t", c=GC, t=2)
                kt8 = kt16[:].rearrange("p c g -> p (c g)").bitcast(FP8) \
                    .rearrange("p (c g t) -> p c g t", c=GC, t=2)

                for ci in range(GC):
                    # S^T[g, h] = sum_w k[g, w] q[h, w]; per gb, accumulate
                    # the two w-parities as plain fp8 matmuls (K=128 each).
                    st = stps.tile([128, 2, 256], F32, name="stps")
                    for gb in range(2):
                        for t in range(2):
                            nc.tensor.matmul(
                                st[:, gb, :],
                                lhsT=kt8[:, ci, gb * 128:(gb + 1) * 128, t],
                                rhs=qt8[:, ci, :, t],
                                start=(t == 0), stop=(t == 1),
                                skip_group_check=True)
                    est = pce.tile([128, 2, 256], FP8, name="est")
                    nc.scalar.activation(est[:], st[:], func=AFT.Exp,
                                         scale=SCALE)
                    # att'[h, w+] = sum_g exp(S^T)[g, h] v'[g, w+]; the ones
                    # column of v' yields the softmax denominator at w=W.
                    avp = aps.tile([128, 2, 512], F32, name="attps")
                    r = pcr.tile([128, 2, 1], F32, name="r")
                    for hb in range(2):
                        nc.tensor.matmul(
                            avp[:, hb, 0:W + 1],
                            lhsT=est[:, :, hb * 128:(hb + 1) * 128],
                            rhs=vt[:, ci, :, :],
                            start=True, stop=True,
                            perf_mode=PM.DoubleRow,
                            skip_group_check=True)
                    nc.vector.reciprocal(r[:], avp[:, :, W:W + 1])
                    for hb in range(2):
                        nc.vector.tensor_scalar_mul(ao[hb][:, ci, :],
                                                    avp[:, hb, 0:W],
                                                    r[:, hb, :])
                for hb in range(2):
                    eng = nc.scalar if g % 2 == 0 else nc.sync
                    eng.dma_start(
                        out=att_v[c0:c0 + GC, hb * 128:(hb + 1) * 128, :]
                        .rearrange("c h w -> h c w"),
                        in_=ao[hb][:])

        # ---------------- phase D: proj conv + bias + residual ----------------
        with tc.tile_pool(name="pd_in", bufs=3) as pdi, \
             tc.tile_pool(name="pd_out", bufs=3) as pdo, \
             tc.tile_pool(name="pd_ps", bufs=2, space="PSUM") as pdp:
            for chunk in range(HW // D_CHUNK):
                off = chunk * D_CHUNK
                ac = pdi.tile([128, D_CHUNK], FP8, name="ac")
                nc.scalar.dma_start(ac[:], att_d[:, off:off + D_CHUNK])
                xc = pdi.tile([128, D_CHUNK], BF16, name="xc")
                nc.scalar.dma_start(xc[:], xbf_d[:, off:off + D_CHUNK])
                pp = pdp.tile([128, D_CHUNK], F32, name="pp")
                for s in range(D_CHUNK // 512):
                    nc.tensor.matmul(pp[:, s * 512:(s + 1) * 512],
                                     lhsT=wproj_sb[:],
                                     rhs=ac[:, s * 512:(s + 1) * 512],
                                     start=True, stop=True)
                xb = pdo.tile([128, D_CHUNK], F32, name="xb")
                nc.scalar.activation(xb[:], pp[:], func=AFT.Identity,
                                     bias=bproj_sb[:], scale=1.0 / WSCALE)
                oc = pdo.tile([128, D_CHUNK], F32, name="oc")
                nc.vector.tensor_add(oc[:], xb[:], xc[:])
                nc.sync.dma_start(out_d[:, off:off + D_CHUNK], oc[:])


_NC_CACHE = [None]
LAST_RESULT = [None]


def _build_nc():
    if _NC_CACHE[0] is None:
        nc = bacc.Bacc("TRN2", target_bir_lowering=False, debug=False,
                       num_devices=8)
        with tile.TileContext(nc) as tc:
            _emit(tc)
        nc.compile()
        _NC_CACHE[0] = nc
    return _NC_CACHE[0]


def kernel(x, condition, Wq, Wkv, bkv, Wproj, bproj):
    x = np.asarray(x, dtype=np.float32)
    condition = np.asarray(condition, dtype=np.float32)
    Wq = np.asarray(Wq, dtype=np.float32)
    Wkv = np.asarray(Wkv, dtype=np.float32)
    bkv = np.asarray(bkv, dtype=np.float32)
    Wproj = np.asarray(Wproj, dtype=np.float32)
    bproj = np.asarray(bproj, dtype=np.float32)

    bf = ml_dtypes.bfloat16
    fp8 = ml_dtypes.float8_e4m3
    wq_h = np.ascontiguousarray(Wq[:, :, 0, 0].T * WSCALE).astype(fp8)
    # [ob, o, ib, i, dy, dx] -> [i, ob, dy, dx, ib, o] -> [128, 18, 2, 128]
    wkv_h = np.ascontiguousarray(
        (Wkv * WSCALE).reshape(2, 128, 2, 128, 3, 3)
        .transpose(3, 0, 4, 5, 2, 1)
    ).reshape(128, 18, 2, 128).astype(fp8)
    bkv_h = np.ascontiguousarray(bkv.reshape(2, 128).T)
    wproj_h = np.ascontiguousarray(Wproj[:, :, 0, 0].T * WSCALE).astype(fp8)
    bproj_h = np.ascontiguousarray(bproj.reshape(C, 1))

    x8_h = x.reshape(B, C, HW).astype(fp8)
    xbf_h = x.reshape(B, C, HW).astype(bf)
    cond8_h = condition.astype(fp8)

    in_maps = []
    for b in range(B):
        in_maps.append({
            "x8": x8_h[b],
            "xbf": xbf_h[b],
            "cond8": cond8_h[b],
            "wq8": wq_h,
            "wkv": wkv_h,
            "bkv": bkv_h,
            "wproj8": wproj_h,
            "bproj": bproj_h,
        })

    nc = _build_nc()
    res = run_bass_kernel_spmd(nc, in_maps, core_ids=list(range(B)))
    LAST_RESULT[0] = res
    out = np.stack([np.asarray(res.results[b]["out"], dtype=np.float32)
                    for b in range(B)])
    return out.reshape(B, C, H, W)


# revision 21
# speedup vs baseline: 1.0520x; 1.0520x over previous
"""Trainium2 Bass kernel for nn_CrossAttention_74818330296332.

Reference computation (per batch b):
  q   = Conv1x1(x, Wq)                          # [C, H, W]
  kv  = Conv3x3_same(condition, Wkv) + bkv      # [2C, H, W]
  k, v = split(kv)                              # each [C, H, W]
  S   = q @ k^T over W (per channel)            # [C, H, H]
  A   = softmax(S * C**-0.5, axis=-1)
  att = A @ v                                   # [C, H, W]
  out = Conv1x1(att, Wproj) + bproj + x

Sharding: data-parallel over batch B=8 across the 8 NeuronCores.

Per-core pipeline (fp32 PSUM accumulate everywhere):
  Host prep: cond/x pre-cast to fp8 (so all device loads are plain HWDGE
    DMAs, no software-DGE casting), weights pre-scaled by 64 into fp8.
  Phase B: 3x3 conv as 9 shifted 1x1 matmuls accumulated in PSUM in
    fp8 DoubleRow (K=256 per matmul).  Spatial chunks of 32 rows with
    1-row halo; q conv (fp8, non-DR) rides along.  Writes q/k/v fp8
    [C, H, W] to DRAM scratch.
  Phase C (attention, per channel-group of 8):  Q^T/K^T loaded via DMA
    transpose of the fp8 scratch viewed as uint16 (so each transposed
    element carries a (w, w+1) fp8 pair == the DoubleRow kt pair);
    K^T repacked on DVE so its kt stride is 16B-aligned for LDWEIGHTS.
    S^T = K Q^T with one DR matmul per 128-g block; exp via ACT into
    fp8; A@v with one DR matmul per 128-h block against V tiles that
    carry a ones-column, so the softmax denominator falls out of the
    same matmul (no FD=1 denominator matmuls); normalization is a
    per-partition scalar multiply split across DVE and GpSimd.
  Phase D: 1x1 proj conv (fp8) + bias + bf16 residual add, streaming.
"""

import os
import sys
import types

import numpy as np
import ml_dtypes

# Make NTFF tracing available if requested (no-op for plain runs).
try:
    import antenv

    if not hasattr(antenv, "axon_hooks"):
        _m = types.ModuleType("antenv.axon_hooks")
        _hook = [None]
        _m.set_axon_ntff_profile_hook = lambda h: _hook.__setitem__(0, h)
        _m.get_axon_ntff_profile_hook = lambda: _hook[0]
        sys.modules["antenv.axon_hooks"] = _m
        antenv.axon_hooks = _m
except Exception:
    pass

import concourse.bass as bass  # noqa: E402
import concourse.tile as tile  # noqa: E402
from concourse import bacc, mybir  # noqa: E402
from concourse.bass_utils import run_bass_kernel_spmd  # noqa: E402

BF16 = mybir.dt.bfloat16
F32 = mybir.dt.float32
FP8 = mybir.dt.float8e4
U16 = mybir.dt.uint16
PM = mybir.MatmulPerfMode
AFT = mybir.ActivationFunctionType

B, C, C_COND, H, W = 8, 128, 256, 256, 256
HW = H * W
SCALE = float(C) ** -0.5
WSCALE = 64.0         # fp8 weight pre-scale (undone in PSUM evacuations)

def _mm_noldw(te, out, rhs, start, stop, perf_mode=None):
    """InstMatmult with no stationary operand: walrus emits no LDWEIGHTS.
    Must directly follow a te.ldweights() of the weights it should use."""
    keep_dims = {0}
    if perf_mode in (mybir.MatmulPerfMode.DoubleRow,
                     mybir.MatmulPerfMode.DoubleRowSwInterleave):
        keep_dims.add(1)
    ifmap_ap = te.lower_ap(rhs.opt(keep_dims), opt=False)
    out_ap = te.lower_ap(out)
    return te.add_instruction(
        mybir.InstMatmult(
            name=te.bass.get_next_instruction_name(),
            replication_resolution=0,
            replication_shift_amnt=0,
            replication_num_rows=0,
            start_tensor_calc=start,
            stop_tensor_calc=stop,
            ins=[ifmap_ap],
            outs=[out_ap],
            perf_mode=perf_mode,
            is_transpose=False,
            ifmap_quant_offset=None,
            weights_quant_offset=None,
            bass_skip_group_check=True,
            tile_position=(0, 0),
            tile_size=(128, 128),
        ))


N_CHUNKS = 8          # phase B spatial chunks
RC = H // N_CHUNKS    # rows per chunk (32)
CW = W + 16           # padded row length (272; %16 for DoubleRow AP strides)
N_GROUPS = 16         # phase C channel groups
GC = C // N_GROUPS    # channels per group (8)
D_CHUNK = 2048        # phase D pixels per chunk


def _emit(tc):
    nc = tc.nc

    x8_d = nc.dram_tensor("x8", [C, HW], FP8, kind="ExternalInput").ap()
    xbf_d = nc.dram_tensor("xbf", [C, HW], BF16, kind="ExternalInput").ap()
    cond_d = nc.dram_tensor("cond8", [C_COND, H, W], FP8,
                            kind="ExternalInput").ap()
    wq_d = nc.dram_tensor("wq8", [C, C], FP8, kind="ExternalInput").ap()
    wkv_d = nc.dram_tensor("wkv", [128, 18, 2, 128], FP8,
                           kind="ExternalInput").ap()
    bkv_d = nc.dram_tensor("bkv", [128, 2], F32, kind="ExternalInput").ap()
    wproj_d = nc.dram_tensor("wproj8", [C, C], FP8, kind="ExternalInput").ap()
    bproj_d = nc.dram_tensor("bproj", [C, 1], F32, kind="ExternalInput").ap()

    q_t = nc.dram_tensor("q_s", [C, H, W], FP8, kind="Internal")
    k_t = nc.dram_tensor("k_s", [C, H, W], FP8, kind="Internal")
    v_t = nc.dram_tensor("v_s", [C, H, W], FP8, kind="Internal")
    att_t = nc.dram_tensor("att_s", [C, HW], FP8, kind="Internal")
    q_d, k_d, v_d, att_d = q_t.ap(), k_t.ap(), v_t.ap(), att_t.ap()
    q16 = q_t.bitcast(U16).ap()     # [C, H, W//2]
    k16 = k_t.bitcast(U16).ap()
    out_d = nc.dram_tensor("out", [C, HW], F32, kind="ExternalOutput").ap()

    q_f = q_d.rearrange("c h w -> c (h w)")
    k_f = k_d.rearrange("c h w -> c (h w)")
    v_f = v_d.rearrange("c h w -> c (h w)")

    # ---------------- globals ----------------
    with tc.tile_pool(name="glob", bufs=1) as glob:
        wproj_sb = glob.tile([128, 128], FP8)
        nc.sync.dma_start(wproj_sb[:], wproj_d[:])
        bproj_sb = glob.tile([128, 1], F32)
        nc.sync.dma_start(bproj_sb[:], bproj_d[:])

        # ---------------- phase B: q conv (fp8) + kv conv (fp8 DoubleRow) ----
        with tc.tile_pool(name="pb_const", bufs=1) as pbc, \
             tc.tile_pool(name="pb_ps", bufs=2, space="PSUM") as cvp, \
             tc.tile_pool(name="pb_stage", bufs=3) as stp, \
             tc.tile_pool(name="pb_x", bufs=2) as xp:
            wq_sb = pbc.tile([128, 128], FP8)
            nc.sync.dma_start(wq_sb[:], wq_d[:])
            # [i, t=(ob,dy,dx), kt=ib, o] fp8, pre-scaled by WSCALE
            wkv_sb = pbc.tile([128, 18, 2, 128], FP8)
            nc.sync.dma_start(wkv_sb[:], wkv_d[:])
            bkv_sb = pbc.tile([128, 2], F32)
            nc.sync.dma_start(bkv_sb[:], bkv_d[:])

            # persistent A/B cond tiles: [128, kt=ib, 34 rows, 272 cols] fp8
            # with zero pad columns 0 and 257.. (w padding of the SAME conv)
            ct = [pbc.tile([128, 2, RC + 2, CW], FP8, name=f"ct{p}")
                  for p in range(2)]
            for p in range(2):
                nc.vector.memset(ct[p][:, :, :, 0:1], 0.0)
                nc.vector.memset(ct[p][:, :, :, W + 1:CW], 0.0)

            for chunk in range(N_CHUNKS):
                r0 = chunk * RC
                par = chunk % 2
                t = ct[par]
                # load cond rows [r0-1, r0+RC+1) with edge clipping
                lo = r0 - 1
                hi = r0 + RC + 1
                tlo = 0
                if lo < 0:
                    nc.vector.memset(t[:, :, 0:1, :], 0.0)
                    lo, tlo = 0, 1
                if hi > H:
                    nc.vector.memset(t[:, :, RC + 1:RC + 2, :], 0.0)
                    hi = H
                for ib in range(2):
                    eng = nc.sync if ib == 0 else nc.scalar
                    eng.dma_start(
                        out=t[:, ib, tlo:tlo + (hi - lo), 1:W + 1],
                        in_=cond_d[ib * 128:(ib + 1) * 128, lo:hi, :])

                # kv conv: 4 quads of 2048 px; per tap one weight feeds 4 MMs
                for quad in range(4):
                    for ob in range(2):
                        ps = cvp.tile([128, 2048], F32, name=f"cv{ob}",
                                      tag="convps")
                        for dy in range(3):
                            for dx in range(3):
                                ti = ob * 9 + dy * 3 + dx
                                for s in range(4):
                                    rr = 8 * quad + 2 * s + dy
                                    nc.tensor.matmul(
                                        ps[:, s * 512:(s + 1) * 512],
                                        lhsT=wkv_sb[:, ti, :, :],
                                        rhs=t[:, :, rr:rr + 2, dx:dx + W],
                                        start=(dy == 0 and dx == 0),
                                        stop=(dy == 2 and dx == 2),
                                        perf_mode=PM.DoubleRow,
                                        skip_group_check=True)
                        kvst = stp.tile([128, 2048], FP8, name="kvst")
                        nc.scalar.activation(kvst[:], ps[:], func=AFT.Identity,
                                             bias=bkv_sb[:, ob:ob + 1],
                                             scale=1.0 / WSCALE)
                        dst = k_f if ob == 0 else v_f
                        off = r0 * W + quad * 2048
                        nc.sync.dma_start(dst[:, off:off + 2048], kvst[:])

                # q conv for the same 32 rows, two halves of 16 rows
                for half in range(2):
                    off = (r0 + 16 * half) * W
                    xt = xp.tile([128, 4096], FP8, name="xt")
                    nc.scalar.dma_start(out=xt[:], in_=x8_d[:, off:off + 4096])
                    qst = stp.tile([128, 4096], FP8, name="qst")
                    for j2 in range(2):
                        qps = cvp.tile([128, 2048], F32, name="qps",
                                       tag="convps")
                        for s in range(4):
                            j = j2 * 4 + s
                            nc.tensor.matmul(
                                qps[:, s * 512:(s + 1) * 512],
                                lhsT=wq_sb[:],
                                rhs=xt[:, j * 512:(j + 1) * 512],
                                start=True, stop=True)
                        nc.vector.tensor_scalar_mul(
                            qst[:, j2 * 2048:(j2 + 1) * 2048], qps[:],
                            1.0 / WSCALE)
                    nc.sync.dma_start(q_f[:, off:off + 4096], qst[:])

        # ---------------- phase C: per-channel attention ----------------
        att_v = att_d.rearrange("c (h w) -> c h w", h=H)
        with tc.tile_pool(name="pc_in", bufs=3) as pci, \
             tc.tile_pool(name="pc_est", bufs=3) as pce, \
             tc.tile_pool(name="pc_r", bufs=3) as pcr, \
             tc.tile_pool(name="pc_ao", bufs=2) as pao, \
             tc.tile_pool(name="pc_stps", bufs=3, space="PSUM") as stps, \
             tc.tile_pool(name="pc_aps", bufs=2, space="PSUM") as aps:
            for g in range(N_GROUPS):
                c0 = g * GC
                # Q^T / K^T via uint16 xbar transpose: each element is an
                # fp8 (w, w+1) pair == the DoubleRow kt pair.
                qt16 = pci.tile([128, GC, H], U16, name="qt16")
                nc.sync.dma_start(
                    out=qt16[:],
                    in_=q16[c0:c0 + GC, :, :].rearrange("c h w -> (c h) w"),
                    transpose=True)
                kt16 = pci.tile([128, GC, H], U16, name="kt16")
                nc.sync.dma_start(
                    out=kt16[:],
                    in_=k16[c0:c0 + GC, :, :].rearrange("c h w -> (c h) w"),
                    transpose=True)
                # V tiles [g0, c, gb, w+ones]: plain permuted loads
                vt = pci.tile([128, GC, 2, W + 1], FP8, name="vt")
                nc.vector.memset(vt[:, :, :, W:W + 1], 1.0)
                for gb in range(2):
                    eng = nc.sync if gb == 0 else nc.scalar
                    eng.dma_start(
                        out=vt[:, :, gb, 0:W],
                        in_=v_d[c0:c0 + GC, gb * 128:(gb + 1) * 128, :]
                        .rearrange("c h w -> h c w"))
                ao = [pao.tile([128, GC, W], FP8, name=f"ao{hb}")
                      for hb in range(2)]

                # fp8 views of the transposed tiles: partition = w-pair,
                # innermost t = w parity (stride 1)
                qt8 = qt16[:].rearrange("p c h -> p (c h)").bitcast(FP8) \
                    .rearrange("p (c h t) -> p c h t", c=GC, t=2)
                kt8 = kt16[:].rearrange("p c g -> p (c g)").bitcast(FP8) \
                    .rearrange("p (c g t) -> p c g t", c=GC, t=2)

                for ci in range(GC):
                    # S^T[g, h] = sum_w k[g, w] q[h, w]; per gb, accumulate
                    # the two w-parities as plain fp8 matmuls (K=128 each).
                    st = stps.tile([128, 2, 256], F32, name="stps")
                    for gb in range(2):
                        for t in range(2):
                            nc.tensor.matmul(
                                st[:, gb, :],
                                lhsT=kt8[:, ci, gb * 128:(gb + 1) * 128, t],
                                rhs=qt8[:, ci, :, t],
                                start=(t == 0), stop=(t == 1),
                                skip_group_check=True)
                    est = pce.tile([128, 2, 256], FP8, name="est")
                    nc.scalar.activation(est[:], st[:], func=AFT.Exp,
                                         scale=SCALE)
                    # att'[h, w+] = sum_g exp(S^T)[g, h] v'[g, w+]; the ones
                    # column of v' yields the softmax denominator at w=W.
                    avp = aps.tile([128, 2, 512], F32, name="attps")
                    r = pcr.tile([128, 2, 1], F32, name="r")
                    for hb in range(2):
                        nc.tensor.matmul(
                            avp[:, hb, 0:W + 1],
                            lhsT=est[:, :, hb * 128:(hb + 1) * 128],
                            rhs=vt[:, ci, :, :],
                            start=True, stop=True,
                            perf_mode=PM.DoubleRow,
                            skip_group_check=True)
                    nc.vector.reciprocal(r[:], avp[:, :, W:W + 1])
                    nc.vector.tensor_scalar_mul(ao[0][:, ci, :],
                                                avp[:, 0, 0:W], r[:, 0, :])
                    nc.scalar.activation(ao[1][:, ci, :], avp[:, 1, 0:W],
                                         func=AFT.Identity, scale=r[:, 1, :])
                for hb in range(2):
                    eng = nc.scalar if g % 2 == 0 else nc.sync
                    eng.dma_start(
                        out=att_v[c0:c0 + GC, hb * 128:(hb + 1) * 128, :]
                        .rearrange("c h w -> h c w"),
                        in_=ao[hb][:])

        # ---------------- phase D: proj conv + bias + residual ----------------
        with tc.tile_pool(name="pd_in", bufs=3) as pdi, \
             tc.tile_pool(name="pd_out", bufs=3) as pdo, \
             tc.tile_pool(name="pd_ps", bufs=2, space="PSUM") as pdp:
            for chunk in range(HW // D_CHUNK):
                off = chunk * D_CHUNK
                ac = pdi.tile([128, D_CHUNK], FP8, name="ac")
                nc.scalar.dma_start(ac[:], att_d[:, off:off + D_CHUNK])
                xc = pdi.tile([128, D_CHUNK], BF16, name="xc")
                nc.sync.dma_start(xc[:], xbf_d[:, off:off + D_CHUNK])
                pp = pdp.tile([128, D_CHUNK], F32, name="pp")
                for s in range(D_CHUNK // 512):
                    nc.tensor.matmul(pp[:, s * 512:(s + 1) * 512],
                                     lhsT=wproj_sb[:],
                                     rhs=ac[:, s * 512:(s + 1) * 512],
                                     start=True, stop=True)
                xb = pdo.tile([128, D_CHUNK], F32, name="xb")
                nc.scalar.activation(xb[:], pp[:], func=AFT.Identity,
                                     bias=bproj_sb[:], scale=1.0 / WSCALE)
                oc = pdo.tile([128, D_CHUNK], F32, name="oc")
                nc.vector.tensor_add(oc[:], xb[:], xc[:])
                nc.sync.dma_start(out_d[:, off:off + D_CHUNK], oc[:])


_NC_CACHE = [None]
LAST_RESULT = [None]


def _build_nc():
    if _NC_CACHE[0] is None:
        nc = bacc.Bacc("TRN2", target_bir_lowering=False, debug=False,
                       num_devices=8)
        with tile.TileContext(nc) as tc:
            _emit(tc)
        nc.compile()
        _NC_CACHE[0] = nc
    return _NC_CACHE[0]


def kernel(x, condition, Wq, Wkv, bkv, Wproj, bproj):
    x = np.asarray(x, dtype=np.float32)
    condition = np.asarray(condition, dtype=np.float32)
    Wq = np.asarray(Wq, dtype=np.float32)
    Wkv = np.asarray(Wkv, dtype=np.float32)
    bkv = np.asarray(bkv, dtype=np.float32)
    Wproj = np.asarray(Wproj, dtype=np.float32)
    bproj = np.asarray(bproj, dtype=np.float32)

    bf = ml_dtypes.bfloat16
    fp8 = ml_dtypes.float8_e4m3
    wq_h = np.ascontiguousarray(Wq[:, :, 0, 0].T * WSCALE).astype(fp8)
    # [ob, o, ib, i, dy, dx] -> [i, ob, dy, dx, ib, o] -> [128, 18, 2, 128]
    wkv_h = np.ascontiguousarray(
        (Wkv * WSCALE).reshape(2, 128, 2, 128, 3, 3)
        .transpose(3, 0, 4, 5, 2, 1)
    ).reshape(128, 18, 2, 128).astype(fp8)
    bkv_h = np.ascontiguousarray(bkv.reshape(2, 128).T)
    wproj_h = np.ascontiguousarray(Wproj[:, :, 0, 0].T * WSCALE).astype(fp8)
    bproj_h = np.ascontiguousarray(bproj.reshape(C, 1))

    x8_h = x.reshape(B, C, HW).astype(fp8)
    xbf_h = x.reshape(B, C, HW).astype(bf)
    cond8_h = condition.astype(fp8)

    in_maps = []
    for b in range(B):
        in_maps.append({
            "x8": x8_h[b],
            "xbf": xbf_h[b],
            "cond8": cond8_h[b],
            "wq8": wq_h,
            "wkv": wkv_h,
            "bkv": bkv_h,
            "wproj8": wproj_h,
            "bproj": bproj_h,
        })

    nc = _build_nc()
    res = run_bass_kernel_spmd(nc, in_maps, core_ids=list(range(B)))
    LAST_RESULT[0] = res
    out = np.stack([np.asarray(res.results[b]["out"], dtype=np.float32)
                    for b in range(B)])
    return out.reshape(B, C, H, W)
